# revision 32
# baseline (speedup 1.0000x reference)
"""Bass/Trainium2 kernel for DirSAGEEmbRes (2-layer directed SAGE + residual).

Strategy (8 NeuronCores, SPMD), T-scatter formulation:
  - dst nodes sharded 1/8 per core; per direction, edges bucketed per
    (core, 512-dst psum bank, src-quarter), sorted by dst within each
    bucket, padded so all cores share one compile-time layout.
  - features pre-transformed before aggregation (z = x0 @ Wl) so gathered
    rows are exactly 128 bf16 = 256B; mean(z[src]) == mean(x0[src]) @ Wl.
  - segment-sum: dma_gather + per-block transposed one-hot matmuls:
    lhsT = gathered 128-edge block (stationary), rhs = narrow one-hot
    ST[e, dstcol] built by one DVE is_equal per block-chunk.  The matmul
    writes msg^T[feat, dst] straight into a scatter PSUM bank — no
    straddle matmuls, no per-window transposes or evictions.
  - 1/deg applied on eviction by a DVE multiply against a broadcast inv
    tile; result injected into the dense-layer PSUM via one
    identity-weight matmul per bank.
  - z / e1 tables exchanged via async AllGather, overlapped with the next
    phase's compute.
"""
import os
import sys

sys.path.insert(0, "/opt/trn_rl_repo")

import numpy as np
import ml_dtypes

import concourse.bass as bass
import concourse.bacc as bacc
import concourse.mybir as mybir
from concourse.tile import TileContext
from concourse.library_config import mlp
from concourse.masks import make_identity

BF = ml_dtypes.bfloat16
NCORE = 8
HID = 128
D0 = 144
BANKC = 512     # dst columns per scatter psum bank
SPAN = 64       # max one-hot width per scatter matmul
KQUEUES = int(os.environ.get("KQUEUES", "4"))
KNOAG = int(os.environ.get("KNOAG", "0"))  # timing probe: skip AllGathers


# ----------------------------------------------------------------- host prep

def _ceil(a, b):
    return -(-a // b)


class PassLayoutT:
    """Compile-time layout of one aggregation pass (shared by all cores).

    Segments are (bank, quarter); mms per segment are
    (block, c0, span, col_id) with col_id indexing the dstcol stream.
    """

    def __init__(self, NBANK):
        self.NBANK = NBANK
        self.seg_info = {}   # (bank, q) -> (idx_pos, seglen, col0)
        self.mms = {}        # (bank, q) -> [(blk, c0, span, col_id)]
        self.tot_idx = 0
        self.tot_col = 0


def _preprocess_t(edge, N, NPC, NPCP, QSH):
    """Per direction: layout + idx/dstcol streams + broadcast inv tile."""
    NBANK = _ceil(NPCP, BANKC)
    src = edge[0].astype(np.int64)
    dst = edge[1].astype(np.int64)
    core = dst // NPC
    ldst = dst - core * NPC
    bank = ldst // BANKC
    col = ldst % BANKC

    deg = np.bincount(core * NPCP + ldst,
                      minlength=NCORE * NPCP).reshape(NCORE, NPCP)
    inv = np.where(deg > 0, 1.0 / np.maximum(deg, 1), 0.0).astype(np.float32)
    inv_b = np.broadcast_to(inv[:, None, :], (NCORE, 128, NPCP)).astype(BF)

    pos = (src // NPC) * NPCP + (src % NPC)
    q = pos // QSH
    iq = (pos - q * QSH).astype(np.int64)

    order = np.lexsort((iq, col, q, bank, core))
    c_s, b_s, q_s, col_s, iq_s = (core[order], bank[order], q[order],
                                  col[order], iq[order])
    key = (c_s * NBANK + b_s) * 4 + q_s
    cnt = np.bincount(key, minlength=NCORE * NBANK * 4).reshape(
        NCORE, NBANK, 4)
    cmax = cnt.max(axis=0)
    assert (cmax > 0).all(), "empty (bank, quarter) segment"
    seglen = np.maximum(_ceil(cmax, 128), 1) * 128          # [NBANK, 4]

    lo = PassLayoutT(NBANK)
    ip = 0
    for b in range(NBANK):
        for qq in range(4):
            # gather only ceil(cmax,16) rows; the 128-rounded tail of the
            # last block is never gathered (stale stage, zero ST weight)
            gn = _ceil(int(cmax[b, qq]), 16) * 16
            lo.seg_info[(b, qq)] = [ip, int(seglen[b, qq]), 0, gn]
            ip += int(seglen[b, qq])
    lo.tot_idx = ip

    # rank within segment
    uniq, first = np.unique(key, return_index=True)
    rank = np.arange(len(key)) - first[np.searchsorted(uniq, key)]
    blk = rank // 128
    eoff = rank % 128

    MAXB = int(seglen.max()) // 128
    gblk = (b_s * 4 + q_s) * MAXB + blk
    NG = NBANK * 4 * MAXB
    colmin = np.full(NG, 1 << 30, np.int64)
    colmax = np.full(NG, -1, np.int64)
    np.minimum.at(colmin, gblk, col_s)
    np.maximum.at(colmax, gblk, col_s)

    # chunk tables + col ids (compile-time, shared across cores)
    col_id = 0
    for b in range(NBANK):
        for qq in range(4):
            lo.seg_info[(b, qq)][2] = col_id
            mlist = []
            nb = int(seglen[b, qq]) // 128
            for k in range(nb):
                gb = (b * 4 + qq) * MAXB + k
                if colmax[gb] < 0:
                    # no edges in this block on any core (trailing pad)
                    continue
                c0 = int(colmin[gb])
                cend = int(colmax[gb]) + 1
                nchunk = _ceil(cend - c0, SPAN)
                for j in range(nchunk):
                    cj = c0 + j * SPAN
                    sp_ = min(SPAN, cend - cj)
                    mlist.append((k, cj, sp_, col_id))
                    col_id += 1
            lo.mms[(b, qq)] = mlist
    lo.tot_col = col_id

    # per-edge stream data
    seg_base = np.zeros((NBANK, 4), np.int64)
    for b in range(NBANK):
        for qq in range(4):
            seg_base[b, qq] = lo.seg_info[(b, qq)][0]
    posn = seg_base[b_s, q_s] + rank

    idx_streams = np.zeros((NCORE, lo.tot_idx), np.int16)
    idx_streams[c_s, posn] = iq_s.astype(np.int16)

    chunk = (col_s - colmin[gblk]) // SPAN
    rel = col_s - (colmin[gblk] + chunk * SPAN)
    # map (gblk, chunk) -> col_id
    colid_lookup = np.full((NG, 512 // SPAN + 1), -1, np.int64)
    for b in range(NBANK):
        for qq in range(4):
            for (k, cj, sp_, cid) in lo.mms[(b, qq)]:
                gb = (b * 4 + qq) * MAXB + k
                j = (cj - colmin[gb]) // SPAN
                colid_lookup[gb, j] = cid
    cids = colid_lookup[gblk, chunk]
    assert (cids >= 0).all()

    dst_streams = np.full((NCORE, lo.tot_col, 128), -1.0, np.float32)
    dst_streams[c_s, cids, eoff] = rel.astype(np.float32)

    idx_wrap = np.ascontiguousarray(
        np.tile(idx_streams.reshape(NCORE, lo.tot_idx // 16, 16)
                .transpose(0, 2, 1), (1, 8, 1)))
    dst_t = np.ascontiguousarray(
        dst_streams.transpose(0, 2, 1))              # [8, 128, tot_col] f32
    return lo, idx_wrap, dst_t, inv_b


# -------------------------------------------------------------- bass builder

def _build_nc(NPC, NPCP, W, QSH, NTH, NBMAX, layouts):
    """layouts: dict[d] -> PassLayoutT"""
    f32 = mybir.dt.float32
    bf16 = mybir.dt.bfloat16
    AF = mybir.ActivationFunctionType
    ALU = mybir.AluOpType
    NBANK = _ceil(NPCP, BANKC)

    nc = bacc.Bacc(None, target_bir_lowering=False, debug=False,
                   num_swdge_queues=KQUEUES)
    t_x0T = nc.dram_tensor("x0T", [128, 2 * NPCP], bf16, kind="ExternalInput")
    t_iota2 = nc.dram_tensor("iota2", [128, 128], bf16, kind="ExternalInput")
    t_idx, t_dst, t_inv = {}, {}, {}
    for d in range(2):
        lo = layouts[d]
        t_idx[d] = nc.dram_tensor(f"idx_{d}", [128, lo.tot_idx // 16],
                                  mybir.dt.int16, kind="ExternalInput")
        t_dst[d] = nc.dram_tensor(f"dst_{d}", [128, lo.tot_col], f32,
                                  kind="ExternalInput")
        t_inv[d] = nc.dram_tensor(f"inv_{d}", [128, NPCP], bf16,
                                  kind="ExternalInput")
    names = ["in1", "in2", "out1", "out2"]
    t_w = {}
    for nm in names:
        d_in = D0 if nm.endswith("1") else HID
        t_w[nm + "_Wl"] = nc.dram_tensor(nm + "_Wl", [d_in, HID], f32, kind="ExternalInput")
        t_w[nm + "_Wr"] = nc.dram_tensor(nm + "_Wr", [d_in, HID], f32, kind="ExternalInput")
        t_w[nm + "_bl"] = nc.dram_tensor(nm + "_bl", [HID], f32, kind="ExternalInput")
    t_lin = nc.dram_tensor("lin_W", [2 * HID], f32, kind="ExternalInput")
    t_linb = nc.dram_tensor("lin_b", [128], f32, kind="ExternalInput")
    t_y = nc.dram_tensor("y", [NPCP], f32, kind="ExternalOutput")

    with TileContext(nc) as tc:
        with (
            tc.tile_pool(name="const", bufs=1) as constp,
            tc.tile_pool(name="stage", bufs=int(os.environ.get("KSB", "4"))) as stagep,
            tc.tile_pool(name="sS", bufs=8) as sp,
            tc.tile_pool(name="small", bufs=3) as smallp,
            tc.tile_pool(name="msg", bufs=3) as msgp,
            tc.tile_pool(name="big", bufs=1) as bigp,
            tc.tile_pool(name="ps", bufs=6, space="PSUM") as psp,
            tc.tile_pool(name="pst", bufs=2, space="PSUM") as pstp,
            tc.tile_pool(name="dram", bufs=1, space="DRAM") as dramp,
        ):
            nc.gpsimd.load_library(mlp)
            # constants
            ident = constp.tile([128, 128], f32, tag="ident")
            make_identity(nc, ident[:])
            ident_bf = constp.tile([128, 128], bf16, tag="identbf")
            nc.vector.tensor_copy(out=ident_bf[:], in_=ident[:])
            iota = constp.tile([128, 128], bf16, tag="iota2")
            nc.sync.dma_start(out=iota[:], in_=t_iota2[:])
            zeros512 = constp.tile([128, 512], bf16, tag="z512")
            nc.vector.memset(zeros512[:], 0.0)
            invt = {}
            for d in range(2):
                invt[d] = constp.tile([128, NPCP], bf16, tag=f"inv{d}",
                                      name=f"inv{d}")
                nc.sync.dma_start(out=invt[d][:], in_=t_inv[d][:])
            # weights: load f32 then convert to bf16 once
            wt = {}
            for nm in names:
                for side in ("Wl", "Wr"):
                    src_t = t_w[nm + "_" + side]
                    a32 = smallp.tile([128, 128], f32, tag="w32")
                    nc.sync.dma_start(out=a32[:], in_=src_t[0:128, :])
                    a = constp.tile([128, 128], bf16, tag=f"{nm}{side}a",
                                    name=f"{nm}{side}a")
                    nc.vector.tensor_copy(out=a[:], in_=a32[:])
                    wt[nm + side + "a"] = a
                    if nm.endswith("1"):
                        b32 = smallp.tile([128, 128], f32, tag="w32")
                        nc.sync.dma_start(out=b32[0:16, :], in_=src_t[128:144, :])
                        b = constp.tile([128, 128], bf16, tag=f"{nm}{side}b",
                                        name=f"{nm}{side}b")
                        nc.vector.tensor_copy(out=b[0:16, :], in_=b32[0:16, :])
                        wt[nm + side + "b"] = b
                bt = constp.tile([128, 1], f32, tag=f"{nm}bl", name=f"{nm}bl")
                nc.sync.dma_start(out=bt[:], in_=t_w[nm + "_bl"][:, None])
                wt[nm + "bl"] = bt
            lin_f = constp.tile([128, 2], f32, tag="linf")
            nc.sync.dma_start(out=lin_f[:], in_=t_lin.rearrange("(h p) -> p h", p=128))
            lin_bf = constp.tile([128, 2], bf16, tag="linbf")
            nc.vector.tensor_copy(out=lin_bf[:], in_=lin_f[:])
            linb_sb = constp.tile([128, 1], f32, tag="linb")
            nc.sync.dma_start(out=linb_sb[:], in_=t_linb[:, None])

            y_sb = constp.tile([128, W], f32, tag="ysb")

            # x0T cached in SBUF once (read 4x by z-passes + layer-1 dense)
            x0sb = constp.tile([128, 2 * NPCP], bf16, tag="x0sb")
            nc.sync.dma_start(out=x0sb[:], in_=t_x0T[:])

            h1T = [bigp.tile([128, NPCP], bf16, tag=f"h1T{d}", name=f"h1T{d}")
                   for d in range(2)]
            z_own = [dramp.tile([NPCP, HID], bf16, tag=f"zown{d}", name=f"zown{d}")
                     for d in range(2)]
            e_own = [dramp.tile([NPCP, HID], bf16, tag=f"eown{d}", name=f"eown{d}")
                     for d in range(2)]
            z_ag = [dramp.tile([NTH, HID], bf16, tag=f"zag{d}", name=f"zag{d}",
                               addr_space="Shared") for d in range(2)]
            e_ag = [dramp.tile([NTH, HID], bf16, tag=f"eag{d}", name=f"eag{d}",
                               addr_space="Shared") for d in range(2)]

            qctr = [0]
            NTILE = _ceil(NPCP, 512)

            # warm the stage pool buffers with finite data: rows skipped by
            # the gather (trailing -1 idx) are read by scatter matmuls with
            # zero weight, and 0 * NaN would poison PSUM
            maxnb = max(max(si[1] for si in layouts[d].seg_info.values())
                        for d in range(2)) // 128
            for _ in range(int(os.environ.get("KSB", "4"))):
                stw = stagep.tile([128, maxnb * 128], bf16, tag="stage")
                nc.vector.memset(stw[:], 0.0)

            def export_tile(srcT, src_off, nn, dram_own, node0):
                """srcT[:, src_off:+nn] (bf16, feature-major) -> node-major DRAM
                rows node0..node0+nn."""
                nwin = nn // 128
                hn = smallp.tile([128, 512], bf16, tag="hn")
                for k in range(nwin):
                    tp = pstp.tile([128, 256], bf16, tag="tr", name="tpbf")
                    nc.tensor.transpose(
                        tp[:, 0:128],
                        srcT[:, src_off + k * 128:src_off + (k + 1) * 128],
                        ident_bf[:])
                    nc.vector.tensor_copy(out=hn[:, k * 128:(k + 1) * 128],
                                          in_=tp[:, 0:128])
                w0 = node0 // 128
                nc.sync.dma_start(
                    out=dram_own[:].rearrange("(w p) j -> p w j", p=128)[:, w0:w0 + nwin, :],
                    in_=hn[:, 0:nn].rearrange("p (w j) -> p w j", j=128))

            def transform_pass(d, lhsT_list, src_rhs, dram_own):
                """out = sum_i lhsT_i^T @ rhs_i per 512-tile, export node-major."""
                for ti in range(NTILE):
                    n0 = ti * 512
                    nn = min(512, NPCP - n0)
                    ps = psp.tile([128, 512], f32, tag="red")
                    rhss = src_rhs(n0, nn)
                    for i, (lhsT, rhs) in enumerate(zip(lhsT_list, rhss)):
                        nc.tensor.matmul(ps[:, 0:nn], lhsT=lhsT, rhs=rhs,
                                         start=(i == 0),
                                         stop=(i == len(lhsT_list) - 1))
                    zt = smallp.tile([128, 512], bf16, tag="zt")
                    nc.scalar.activation(out=zt[:, 0:nn], in_=ps[:, 0:nn],
                                         func=AF.Copy)
                    export_tile(zt, 0, nn, dram_own, n0)

            def load_xa(n0, nn):
                return (x0sb[:, n0:n0 + nn],
                        x0sb[0:16, NPCP + n0:NPCP + n0 + nn])

            def reduce_dense(d, table_ap, dense_mms, bias, out_cb):
                """T-scatter aggregation pass + fused dense layer."""
                lo = layouts[d]
                for b in range(NBANK):
                    n0 = b * BANKC
                    nn = min(BANKC, NPCP - n0)
                    psS = psp.tile([128, 512], f32, tag="red",
                                   name=f"agg{d}_{b}")
                    # zero the scatter bank on ACT; all scatter matmuls can
                    # then run start=False (accumulate-onto-zeros)
                    nc.scalar.activation(out=psS[:, 0:512], in_=zeros512[:],
                                         func=AF.Copy)
                    total = sum(len(lo.mms[(b, qq)]) for qq in range(4))
                    seen = 0
                    for qq in range(4):
                        ip, seglen, dc0, gn = lo.seg_info[(b, qq)]
                        mlist = lo.mms[(b, qq)]
                        nb = seglen // 128
                        ncols = len(mlist)
                        idxs = smallp.tile([128, max(seglen // 16, 8)],
                                           mybir.dt.int16, tag="idxs")
                        nc.sync.dma_start(
                            out=idxs[:, 0:seglen // 16],
                            in_=t_idx[d][:, ip // 16: ip // 16 + seglen // 16])
                        dstc = smallp.tile([128, max(ncols, 1)], f32,
                                           tag="dstc")
                        if ncols:
                            nc.sync.dma_start(
                                out=dstc[:, 0:ncols],
                                in_=t_dst[d][:, dc0:dc0 + ncols])
                        stage = stagep.tile([128, nb * 128], bf16, tag="stage")
                        nc.gpsimd.dma_gather(
                            stage[:].rearrange("p (b e) -> p b e", e=128),
                            table_ap[qq * QSH:(qq + 1) * QSH],
                            idxs[:, 0:gn // 16],
                            gn, gn, 128,
                            single_packet=(os.environ.get("KSP", "0") == "1"),
                            queue_num=qctr[0] % KQUEUES,
                        )
                        qctr[0] += 1
                        for mi, (blk, c0, span, cid) in enumerate(mlist):
                            ST = sp.tile([128, SPAN], bf16, tag="ST")
                            nc.vector.tensor_scalar(
                                out=ST[:, 0:span], in0=iota[:, 0:span],
                                scalar1=dstc[:, cid - dc0:cid - dc0 + 1],
                                scalar2=None, op0=ALU.is_equal)
                            seen += 1
                            nc.tensor.matmul(
                                psS[:, c0:c0 + span],
                                lhsT=stage[:, blk * 128:(blk + 1) * 128],
                                rhs=ST[:, 0:span],
                                start=False,
                                stop=(seen == total))
                    # eviction: scale by 1/deg (broadcast inv tile), inject
                    # into the dense psum via one identity matmul
                    msgS = msgp.tile([128, 512], bf16, tag="msgS")
                    nc.vector.tensor_tensor(
                        out=msgS[:, 0:nn], in0=psS[:, 0:nn],
                        in1=invt[d][:, n0:n0 + nn], op=ALU.mult)
                    ps = psp.tile([128, 512], f32, tag="red",
                                  name=f"dense{d}_{b}")
                    dm = dense_mms(n0, nn)
                    for i, (lhsT, rhs) in enumerate(dm):
                        nc.tensor.matmul(ps[:, 0:nn], lhsT=lhsT, rhs=rhs,
                                         start=(i == 0), stop=False)
                    nc.tensor.matmul(ps[:, 0:nn], lhsT=ident_bf[:],
                                     rhs=msgS[:, 0:nn],
                                     start=False, stop=True)
                    out_cb(ps, n0, nn)

            for d in range(2):
                nm1, nm2 = names[2 * d], names[2 * d + 1]
                # ---- z-pass: z = x0 @ Wl1, export + AllGather ----
                transform_pass(
                    d,
                    [wt[nm1 + "Wla"][:], wt[nm1 + "Wlb"][0:16, :]],
                    lambda n0, nn: list(load_xa(n0, nn)),
                    z_own[d])
                if not KNOAG:
                    nc.gpsimd.collective_compute(
                        "AllGather", mybir.AluOpType.bypass,
                        replica_groups=[list(range(NCORE))],
                        ins=[z_own[d][:]], outs=[z_ag[d][:]])

            for d in range(2):
                nm1, nm2 = names[2 * d], names[2 * d + 1]

                # ---- layer 1: mean(z) + x0 @ Wr1, relu -> h1T ----
                def l1_dense(n0, nn):
                    xa, xb = load_xa(n0, nn)
                    return [(wt[nm1 + "Wra"][:], xa),
                            (wt[nm1 + "Wrb"][0:16, :], xb)]

                def l1_out(ps, n0, nn, d=d, nm1=nm1):
                    nc.scalar.activation(out=h1T[d][:, n0:n0 + nn],
                                         in_=ps[:, 0:nn],
                                         func=AF.Relu,
                                         bias=wt[nm1 + "bl"][:, 0:1], scale=1.0)

                reduce_dense(d, z_ag[d][:], l1_dense, wt[nm1 + "bl"], l1_out)

                # ---- e1-pass: e1 = h1 @ Wl2, export + AllGather ----
                transform_pass(
                    d,
                    [wt[nm2 + "Wla"][:]],
                    lambda n0, nn: [h1T[d][:, n0:n0 + nn]],
                    e_own[d])
                if not KNOAG:
                    nc.gpsimd.collective_compute(
                        "AllGather", mybir.AluOpType.bypass,
                        replica_groups=[list(range(NCORE))],
                        ins=[e_own[d][:]], outs=[e_ag[d][:]])

            for d in range(2):
                nm1, nm2 = names[2 * d], names[2 * d + 1]

                # ---- layer 2: mean(e1) + h1 @ Wr2, relu, residual ----
                def l2_dense(n0, nn, d=d, nm2=nm2):
                    return [(wt[nm2 + "Wra"][:], h1T[d][:, n0:n0 + nn])]

                def l2_out(ps, n0, nn, d=d, nm2=nm2):
                    h2t = smallp.tile([128, 512], bf16, tag="h2t")
                    nc.scalar.activation(out=h2t[:, 0:nn], in_=ps[:, 0:nn],
                                         func=AF.Relu,
                                         bias=wt[nm2 + "bl"][:, 0:1], scale=1.0)
                    nc.vector.tensor_add(out=h1T[d][:, n0:n0 + nn],
                                         in0=h1T[d][:, n0:n0 + nn],
                                         in1=h2t[:, 0:nn])

                reduce_dense(d, e_ag[d][:], l2_dense, wt[nm2 + "bl"], l2_out)

            # ---------------- y = h_in @ lin[:128] + h_out @ lin[128:] + b ----
            for wv in range(W):
                yp = psp.tile([128, 512], f32, tag="red", name="yp")
                nc.tensor.matmul(yp[:, 0:1],
                                 lhsT=h1T[0][:, wv * 128:(wv + 1) * 128],
                                 rhs=lin_bf[:, 0:1], start=True, stop=False)
                nc.tensor.matmul(yp[:, 0:1],
                                 lhsT=h1T[1][:, wv * 128:(wv + 1) * 128],
                                 rhs=lin_bf[:, 1:2], start=False, stop=True)
                nc.scalar.activation(out=y_sb[:, wv:wv + 1], in_=yp[:, 0:1],
                                     func=AF.Copy)
            nc.vector.tensor_scalar(
                out=y_sb[:], in0=y_sb[:],
                scalar1=linb_sb[:, 0:1], scalar2=None,
                op0=ALU.add)
            nc.sync.dma_start(out=t_y.rearrange("(w p) -> p w", p=128), in_=y_sb[:])

    nc.compile()
    return nc


# ------------------------------------------------------------------ wrapper

def _prep_all(x, edge_in, edge_out, emb):
    N = x.shape[0]
    NPC = N // NCORE
    W = _ceil(NPC, 128)
    NPCP = W * 128
    QSH = 2 * NPCP
    NTH = NCORE * NPCP

    x = np.asarray(x, np.float32)
    emb = np.asarray(emb, np.float32)

    pre = {}
    for d, edge in enumerate((edge_in, edge_out)):
        pre[d] = _preprocess_t(np.asarray(edge), N, NPC, NPCP, QSH)

    NBMAX = 2
    iota2 = np.broadcast_to(np.arange(128, dtype=np.float32),
                            (128, 128)).astype(BF)

    x0T = np.zeros((NCORE, 128, 2 * NPCP), np.float32)
    for c in range(NCORE):
        blk = np.zeros((NPCP, D0), np.float32)
        blk[:NPC, 0:128] = x[c * NPC:(c + 1) * NPC]
        blk[:NPC, 128:144] = emb[c * NPC:(c + 1) * NPC]
        x0T[c, :, :NPCP] = blk[:, 0:128].T
        x0T[c, 0:16, NPCP:] = blk[:, 128:144].T
    x0T = x0T.astype(BF)

    layouts = {d: pre[d][0] for d in range(2)}
    dims = dict(N=N, NPC=NPC, NPCP=NPCP, W=W, QSH=QSH, NTH=NTH, NBMAX=NBMAX)
    return dims, layouts, pre, x0T, iota2


def _in_maps(dims, pre, x0T, iota2, kw):
    maps = []
    for c in range(NCORE):
        m = {"x0T": np.ascontiguousarray(x0T[c]), "iota2": np.asarray(iota2)}
        for d in range(2):
            _, idx_wrap, dst_t, inv_b = pre[d]
            m[f"idx_{d}"] = np.ascontiguousarray(idx_wrap[c])
            m[f"dst_{d}"] = np.ascontiguousarray(dst_t[c])
            m[f"inv_{d}"] = np.ascontiguousarray(inv_b[c])
        for nm in ("in1", "in2", "out1", "out2"):
            m[nm + "_Wl"] = np.asarray(kw[nm + "_Wl"], np.float32)
            m[nm + "_Wr"] = np.asarray(kw[nm + "_Wr"], np.float32)
            m[nm + "_bl"] = np.asarray(kw[nm + "_bl"], np.float32)
        m["lin_W"] = np.asarray(kw["lin_W"], np.float32).reshape(-1)
        m["lin_b"] = np.full(128, np.asarray(kw["lin_b"], np.float32).reshape(-1)[0], np.float32)
        maps.append(m)
    return maps


def kernel(x, edge_in, edge_out, emb, **kw):
    from concourse.bass_utils import run_bass_kernel_spmd
    dims, layouts, pre, x0T, iota2 = _prep_all(x, edge_in, edge_out, emb)
    nc = _build_nc(dims["NPC"], dims["NPCP"], dims["W"], dims["QSH"],
                   dims["NTH"], dims["NBMAX"], layouts)
    maps = _in_maps(dims, pre, x0T, iota2, kw)
    res = run_bass_kernel_spmd(nc, maps, core_ids=list(range(NCORE)))
    NPC = dims["NPC"]
    y = np.empty(dims["N"], np.float32)
    for c in range(NCORE):
        y[c * NPC:(c + 1) * NPC] = res.results[c]["y"][:NPC]
    return y
